# revision 1
# baseline (speedup 1.0000x reference)
"""Cross-attention Trainium2 Bass kernel.

Reference computation (per batch b):
  q = x @ Wq;  k = ctx @ Wk;  v = ctx @ Wv        (16 heads x 64 dim)
  sim = q k^T / 8;  attn = softmax(sim);  out = (attn v) @ Wo + bo

Sharding: 8 cores = 4 batches x 2 head-groups (8 heads each).
Each core computes a partial output [2048, 1024] (its 8 heads' contribution
through Wo); the host sums the two partials per batch and adds the bias.

Per-core data layout (host-prepared; matmul operands cast to bf16 on host
— measured rel err 2.4e-3 vs the fp32 reference, ~30% faster than the
float32r variant which does not reach full PE rate at moving dim 256):
  xT   [1024, 2048]  = x[b].T               (contraction dim on partitions)
  ctxT [ 768, 2048]  = context[b].T
  wq   [1024,  512]  = Wq[:, g*512:+512] * 0.125   (attn scale folded in)
  wk   [ 768,  512]  = Wk[:, g*512:+512]
  wv   [ 768,  512]  = Wv[:, g*512:+512]
  wo   [ 512, 1024]  = Wo[g*512:+512, :]

Device pipeline:
  A: Q^T [512, 2048] = wq^T @ xT        (psum; inner dim on partitions)
  B: K^T [512, 2048] = wk^T @ ctxT ;  V [2048, 520] = ctxT^T @ wv
     (V natural orientation; per-head 65th column set to 1.0 so that the
      P'@V_ext matmul also produces the softmax denominators)
  C: per head pair (row-packed K=64 matmuls) and 256-wide query block:
       S^T [kv, nq] = K^T.T @ Q^T   (scores; no max-subtraction needed:
                                     |S| <= ~3 by construction)
       P'^T = exp(S^T)              (ACT, fused PSUM->SBUF)
       O'^T [65, nq] = V_ext.T @ P'^T   (row 64 = sum_j P' = denominators)
       O^T = O'^T[0:64] * broadcast(1/O'^T[64])
  D: out [2048, 1024] = O^T.T @ wo      (partial; host adds pair + bias)
"""
import sys

sys.path.insert(0, "/opt/trn_rl_repo")

import numpy as np

import concourse.bass as bass  # noqa: F401  (bass types used via tile/bacc)
import concourse.tile as tile
from concourse import bacc, mybir
from concourse import bass_utils

# Problem constants (hardcoded per harness contract).
B = 4
NQ = 2048
NKV = 2048
IN_DIM = 1024
CTX_DIM = 768
N_HEADS = 16
HEAD_DIM = 64
G = 512          # inner dim per core (8 heads)
HPC = 8          # heads per core
OUT_DIM = 1024
SCALE = HEAD_DIM ** -0.5

NQB = 256        # query block width (measured faster than 512: better
                 # ACT/PE pipelining at the same exp-instruction shape)
NQBLKS = NQ // NQB            # 8
KVC = NKV // 128              # 16 kv chunks
VW = HEAD_DIM + 1             # 65: V columns per head incl. ones column
PACK_S = True                 # row-pack head pairs in the S matmul (K=64)

_CACHE = {}


def _build_program(pack_s=PACK_S, reps=1, mmdt="float32r", nqb=None):
    if nqb is None:
        nqb = NQB
    nqblks = NQ // nqb
    cpt = 1024 // nqb  # S-psum tile holds cpt kv-chunks of width nqb
    f32 = mybir.dt.float32
    f32r = getattr(mybir.dt, mmdt)
    EXP = mybir.ActivationFunctionType.Exp

    nc = bacc.Bacc("TRN2", target_bir_lowering=False, debug=False,
                   enable_asserts=False, num_devices=8)
    xT_d = nc.dram_tensor("xT", [IN_DIM, NQ], f32r, kind="ExternalInput").ap()
    ctxT_d = nc.dram_tensor("ctxT", [CTX_DIM, NKV], f32r, kind="ExternalInput").ap()
    wq_d = nc.dram_tensor("wq", [IN_DIM, G], f32r, kind="ExternalInput").ap()
    wk_d = nc.dram_tensor("wk", [CTX_DIM, G], f32r, kind="ExternalInput").ap()
    wv_d = nc.dram_tensor("wv", [CTX_DIM, G], f32r, kind="ExternalInput").ap()
    wo_d = nc.dram_tensor("wo", [G, OUT_DIM], f32r, kind="ExternalInput").ap()
    out_d = nc.dram_tensor("out", [NQ, OUT_DIM], f32, kind="ExternalOutput").ap()

    KQ = IN_DIM // 128   # 8 contraction chunks for Q proj
    KC = CTX_DIM // 128  # 6 contraction chunks for K/V proj
    MC = G // 128        # 4 inner chunks (head pairs)

    from contextlib import ExitStack

    def _emit(tc):
        with ExitStack() as ctx:
            # Persistent tensors (live across phases).
            pQT = ctx.enter_context(tc.tile_pool(name="qt", bufs=1))
            pKT = ctx.enter_context(tc.tile_pool(name="kt", bufs=1))
            pV = ctx.enter_context(tc.tile_pool(name="vv", bufs=1))
            QT = pQT.tile([128, MC * NQ], f32r)    # chunk m at free [m*NQ, (m+1)*NQ)
            KT = pKT.tile([128, MC * NKV], f32r)
            V = pV.tile([128, KVC * HPC * VW], f32r)  # chunk kvc at [kvc*520, +520)

            # --- Phase A: Q^T = wq^T @ xT ---------------------------------
            with tc.tile_pool(name="xt", bufs=1) as pxT, \
                 tc.tile_pool(name="wq", bufs=1) as pwq, \
                 tc.tile_pool(name="psA", bufs=4, space="PSUM") as psA:
                xT = pxT.tile([128, KQ * NQ], f32r)
                nc.sync.dma_start(
                    xT[:].rearrange("p (c n) -> p c n", c=KQ),
                    xT_d.rearrange("(c p) n -> p c n", p=128))
                wq = pwq.tile([128, KQ * G], f32r)
                nc.sync.dma_start(
                    wq[:].rearrange("p (c n) -> p c n", c=KQ),
                    wq_d.rearrange("(c p) n -> p c n", p=128))
                for m in range(MC):
                    for q in range(NQ // 512):
                        ps = psA.tile([128, 512], f32)
                        for k in range(KQ):
                            nc.tensor.matmul(
                                ps[:],
                                wq[:, k * G + m * 128:k * G + (m + 1) * 128],
                                xT[:, k * NQ + q * 512:k * NQ + (q + 1) * 512],
                                start=(k == 0), stop=(k == KQ - 1))
                        nc.vector.tensor_copy(
                            QT[:, m * NQ + q * 512:m * NQ + (q + 1) * 512], ps[:])

            # --- Phase B: K^T = wk^T @ ctxT ; V = ctxT^T @ wv -------------
            with tc.tile_pool(name="ct", bufs=1) as pcT, \
                 tc.tile_pool(name="wk", bufs=1) as pwk, \
                 tc.tile_pool(name="wv", bufs=1) as pwv, \
                 tc.tile_pool(name="psB", bufs=4, space="PSUM") as psB:
                ctxT = pcT.tile([128, KC * NKV], f32r)
                nc.sync.dma_start(
                    ctxT[:].rearrange("p (c n) -> p c n", c=KC),
                    ctxT_d.rearrange("(c p) n -> p c n", p=128))
                wk = pwk.tile([128, KC * G], f32r)
                nc.sync.dma_start(
                    wk[:].rearrange("p (c n) -> p c n", c=KC),
                    wk_d.rearrange("(c p) n -> p c n", p=128))
                wv = pwv.tile([128, KC * G], f32r)
                nc.sync.dma_start(
                    wv[:].rearrange("p (c n) -> p c n", c=KC),
                    wv_d.rearrange("(c p) n -> p c n", p=128))
                for m in range(MC):
                    for q in range(NKV // 512):
                        ps = psB.tile([128, 512], f32, tag="pskt")
                        for k in range(KC):
                            nc.tensor.matmul(
                                ps[:],
                                wk[:, k * G + m * 128:k * G + (m + 1) * 128],
                                ctxT[:, k * NKV + q * 512:k * NKV + (q + 1) * 512],
                                start=(k == 0), stop=(k == KC - 1))
                        nc.vector.tensor_copy(
                            KT[:, m * NKV + q * 512:m * NKV + (q + 1) * 512], ps[:])
                # ones columns for the denominator trick (memset rejects
                # float32r, so write those bits through a float32 view)
                ones_view = V[:].bitcast(f32) if mmdt == "float32r" else V[:]
                nc.gpsimd.memset(
                    ones_view
                    .rearrange("p (c h e) -> p c h e", c=KVC, e=VW)[:, :, :, 64:65],
                    1.0)
                for kvc in range(KVC):
                    ps = psB.tile([128, 512], f32, tag="psv")
                    for k in range(KC):
                        nc.tensor.matmul(
                            ps[:],
                            ctxT[:, k * NKV + kvc * 128:k * NKV + (kvc + 1) * 128],
                            wv[:, k * G:(k + 1) * G],
                            start=(k == 0), stop=(k == KC - 1))
                    nc.vector.tensor_copy(
                        V[:, kvc * HPC * VW:(kvc + 1) * HPC * VW]
                        .rearrange("p (h e) -> p h e", e=VW)[:, :, 0:64],
                        ps[:].rearrange("p (h e) -> p h e", e=64))

            # --- Phase C: attention ---------------------------------------
            # OT allocated here (not earlier) to keep phase A/B under the
            # SBUF cap; it persists through phase D via the outer ExitStack.
            pOT = ctx.enter_context(tc.tile_pool(name="ot", bufs=1))
            OT = pOT.tile([128, MC * NQ], f32r)
            nquads = 2 * (KVC // cpt)  # live P' tiles per (pair, q-block)
            with tc.tile_pool(name="pq", bufs=nquads + 2) as pP, \
                 tc.tile_pool(name="den", bufs=4) as pDen, \
                 tc.tile_pool(name="psS", bufs=3, space="PSUM") as psS, \
                 tc.tile_pool(name="psO", bufs=2, space="PSUM") as psO:
                for m in range(MC):
                    for q in range(nqblks):
                        qo = q * nqb
                        quads = ([], [])  # P' tiles for h1, h2
                        for kvg in range(KVC // cpt):
                            s1 = psS.tile([128, cpt * nqb], f32, tag="s")
                            s2 = psS.tile([128, cpt * nqb], f32, tag="s")
                            for j in range(cpt):
                                kvc = kvg * cpt + j
                                ko = m * NKV + kvc * 128
                                kw = dict(start=True, stop=True)
                                if pack_s:
                                    kw1 = dict(tile_position=(0, 0), **kw)
                                    kw2 = dict(tile_position=(64, 0), **kw)
                                else:
                                    kw1 = kw2 = kw
                                nc.tensor.matmul(
                                    s1[:, j * nqb:(j + 1) * nqb],
                                    KT[0:64, ko:ko + 128],
                                    QT[0:64, m * NQ + qo:m * NQ + qo + nqb], **kw1)
                                nc.tensor.matmul(
                                    s2[:, j * nqb:(j + 1) * nqb],
                                    KT[64:128, ko:ko + 128],
                                    QT[64:128, m * NQ + qo:m * NQ + qo + nqb], **kw2)
                            for hi, s in ((0, s1), (1, s2)):
                                pquad = pP.tile([128, cpt * nqb], f32r, tag="pq")
                                nc.scalar.activation(pquad[:], s[:], EXP)
                                quads[hi].append(pquad)
                        for hi in range(2):
                            h = 2 * m + hi
                            po = psO.tile([VW, nqb], f32)
                            for kvc in range(KVC):
                                nc.tensor.matmul(
                                    po[:],
                                    V[:, kvc * HPC * VW + h * VW:
                                       kvc * HPC * VW + (h + 1) * VW],
                                    quads[hi][kvc // cpt][:, (kvc % cpt) * nqb:
                                                          (kvc % cpt + 1) * nqb],
                                    start=(kvc == 0), stop=(kvc == KVC - 1))
                            d = pDen.tile([1, nqb], f32, tag="d")
                            nc.vector.reciprocal(d[:], po[64:65, :])
                            R = pDen.tile([64, nqb], f32, tag="r")
                            nc.gpsimd.partition_broadcast(R[:], d[:])
                            nc.vector.tensor_mul(
                                OT[hi * 64:(hi + 1) * 64, m * NQ + qo:m * NQ + qo + nqb],
                                po[0:64, :], R[:])

            # --- Phase D: out = O^T.T @ wo --------------------------------
            with tc.tile_pool(name="wo", bufs=1) as pwo, \
                 tc.tile_pool(name="outst", bufs=3) as pOut, \
                 tc.tile_pool(name="psD", bufs=3, space="PSUM") as psD:
                wo = pwo.tile([128, MC * OUT_DIM], f32r)
                nc.sync.dma_start(
                    wo[:].rearrange("p (c n) -> p c n", c=MC),
                    wo_d.rearrange("(c p) n -> p c n", p=128))
                for mq in range(NQ // 128):
                    for n2 in range(OUT_DIM // 512):
                        ps = psD.tile([128, 512], f32)
                        for c in range(MC):
                            nc.tensor.matmul(
                                ps[:],
                                OT[:, c * NQ + mq * 128:c * NQ + (mq + 1) * 128],
                                wo[:, c * OUT_DIM + n2 * 512:c * OUT_DIM + (n2 + 1) * 512],
                                start=(c == 0), stop=(c == MC - 1))
                        ob = pOut.tile([128, 512], f32)
                        nc.vector.tensor_copy(ob[:], ps[:])
                        nc.sync.dma_start(
                            out_d[mq * 128:(mq + 1) * 128, n2 * 512:(n2 + 1) * 512],
                            ob[:])

    with tile.TileContext(nc, trace_sim=False) as tc:
        if reps == 1:
            _emit(tc)
        else:
            with tc.For_i(0, reps, 1):
                _emit(tc)

    nc.compile()
    return nc


def get_program(pack_s=PACK_S, reps=1, mmdt="bfloat16", nqb=None):
    key = ("prog", pack_s, reps, mmdt, nqb)
    if key not in _CACHE:
        _CACHE[key] = _build_program(pack_s, reps, mmdt, nqb)
    return _CACHE[key]


def make_in_maps(x, context, Wq, Wk, Wv, Wo, mmdt="bfloat16"):
    import ml_dtypes
    hdt = np.float32 if mmdt == "float32r" else np.dtype(ml_dtypes.bfloat16)
    x = np.asarray(x, dtype=np.float32)
    context = np.asarray(context, dtype=np.float32)
    Wq = np.asarray(Wq, dtype=np.float32)
    Wk = np.asarray(Wk, dtype=np.float32)
    Wv = np.asarray(Wv, dtype=np.float32)
    Wo = np.asarray(Wo, dtype=np.float32)
    xT = [np.ascontiguousarray(x[b].T).astype(hdt) for b in range(B)]
    ctxT = [np.ascontiguousarray(context[b].T).astype(hdt) for b in range(B)]
    wq = [(np.ascontiguousarray(Wq[:, g * G:(g + 1) * G]) * np.float32(SCALE))
          .astype(hdt) for g in range(2)]
    wk = [np.ascontiguousarray(Wk[:, g * G:(g + 1) * G]).astype(hdt) for g in range(2)]
    wv = [np.ascontiguousarray(Wv[:, g * G:(g + 1) * G]).astype(hdt) for g in range(2)]
    wo = [np.ascontiguousarray(Wo[g * G:(g + 1) * G, :]).astype(hdt) for g in range(2)]
    in_maps = []
    for c in range(8):
        b, g = c // 2, c % 2
        in_maps.append({"xT": xT[b], "ctxT": ctxT[b], "wq": wq[g],
                        "wk": wk[g], "wv": wv[g], "wo": wo[g]})
    return in_maps


def run_device(nc, in_maps):
    return bass_utils.run_bass_kernel_spmd(nc, in_maps, core_ids=list(range(8)))


def kernel(x, context, Wq, Wk, Wv, Wo, bo, mmdt="bfloat16"):
    nc = get_program(mmdt=mmdt)
    in_maps = make_in_maps(x, context, Wq, Wk, Wv, Wo, mmdt=mmdt)
    res = run_device(nc, in_maps)
    bo = np.asarray(bo, dtype=np.float32)
    out = np.empty((B, NQ, OUT_DIM), dtype=np.float32)
    for b in range(B):
        out[b] = res.results[2 * b]["out"] + res.results[2 * b + 1]["out"] + bo
    return out



# revision 2
# speedup vs baseline: 1.0316x; 1.0316x over previous
"""Cross-attention Trainium2 Bass kernel (optimized).

Reference computation (per batch b):
  q = x @ Wq;  k = ctx @ Wk;  v = ctx @ Wv        (16 heads x 64 dim)
  sim = q k^T / 8;  attn = softmax(sim);  out = (attn v) @ Wo + bo

Sharding: 8 cores = 4 batches x 2 head-groups (8 heads each).
Each core computes a partial output [2048, 1024] (its 8 heads' contribution
through Wo); the host sums the two partials per batch and adds the bias.

Per-core data layout (host-prepared, bf16 matmul operands):
  xT   [1024, 2048]  = x[b].T               (contraction dim on partitions)
  ctxT [ 768, 2048]  = context[b].T
  wq   [1024,  512]  = Wq[:, g*512:+512] * 0.125   (attn scale folded in)
  wk   [ 768,  512]  = Wk[:, g*512:+512]
  wv   [ 768,  512]  = Wv[:, g*512:+512]
  wo   [ 512, 1024]  = Wo[g*512:+512, :]

Device pipeline (single TileContext; phases interleaved per head-pair m so
the tensor engine always has projection work to fill softmax-exp waits —
the ACT engine's 33.5M exps/core are the serial floor):
  A(m): Q^T chunk = wq^T @ xT
  B(m): K^T chunk = wk^T @ ctxT;  V = ctxT^T @ wv (once, emitted under C(0)
        so its matmuls fill PE idle during the first exp groups; per-head
        65th V column = 1.0 so attn@V also produces softmax denominators)
  C(m): software-pipelined per 512-wide query block: per kv-group emit the
        row-packed S^T matmul pair (2 heads at tile_position 0/64, K=64),
        the two exps (PSUM->SBUF bf16), then the attn@V chunk matmuls of
        the previous kv-group; normalize by the broadcast reciprocal of
        the denominator row.
  D: out = O^T.T @ wo  (partial; host adds pair + bias)
"""
import sys

sys.path.insert(0, "/opt/trn_rl_repo")

import numpy as np

import concourse.bass as bass  # noqa: F401
import concourse.tile as tile
from concourse import bacc, mybir
from concourse import bass_utils

B = 4
NQ = 2048
NKV = 2048
IN_DIM = 1024
CTX_DIM = 768
N_HEADS = 16
HEAD_DIM = 64
G = 512
HPC = 8
OUT_DIM = 1024
SCALE = HEAD_DIM ** -0.5
KVC = NKV // 128
VW = HEAD_DIM + 1
NQB = 512
CPT = 1024 // NQB      # kv chunks per S-psum tile (tile = 2 PSUM banks)

PSS_BUFS = 3
PSAB_BUFS = 1
PSO_BUFS = 1
PQ_BUFS = 2 * (KVC // CPT) + 2

_CACHE = {}


def _build_program(reps=1, mmdt="bfloat16"):
    f32 = mybir.dt.float32
    hdt = getattr(mybir.dt, mmdt)
    EXP = mybir.ActivationFunctionType.Exp
    nqb = NQB
    cpt = CPT
    nqblks = NQ // nqb
    KQ = IN_DIM // 128
    KC = CTX_DIM // 128
    MC = G // 128

    nc = bacc.Bacc("TRN2", target_bir_lowering=False, debug=False,
                   enable_asserts=False, num_devices=8)
    xT_d = nc.dram_tensor("xT", [IN_DIM, NQ], hdt, kind="ExternalInput").ap()
    ctxT_d = nc.dram_tensor("ctxT", [CTX_DIM, NKV], hdt, kind="ExternalInput").ap()
    wq_d = nc.dram_tensor("wq", [IN_DIM, G], hdt, kind="ExternalInput").ap()
    wk_d = nc.dram_tensor("wk", [CTX_DIM, G], hdt, kind="ExternalInput").ap()
    wv_d = nc.dram_tensor("wv", [CTX_DIM, G], hdt, kind="ExternalInput").ap()
    wo_d = nc.dram_tensor("wo", [G, OUT_DIM], hdt, kind="ExternalInput").ap()
    out_d = nc.dram_tensor("out", [NQ, OUT_DIM], f32, kind="ExternalOutput").ap()

    from contextlib import ExitStack

    def _emit(tc):
        with ExitStack() as ctx:
            pQT = ctx.enter_context(tc.tile_pool(name="qt", bufs=1))
            pKT = ctx.enter_context(tc.tile_pool(name="kt", bufs=1))
            pV = ctx.enter_context(tc.tile_pool(name="vv", bufs=1))
            pOT = ctx.enter_context(tc.tile_pool(name="ot", bufs=1))
            pIN = ctx.enter_context(tc.tile_pool(name="ins", bufs=1))
            QT = pQT.tile([128, MC * NQ], hdt)
            KT = pKT.tile([128, MC * NKV], hdt)
            V = pV.tile([128, KVC * HPC * VW], hdt)
            OT = pOT.tile([128, MC * NQ], hdt)

            # input DMAs, chunk-split so the first projections start early
            wq = pIN.tile([128, KQ * G], hdt, tag="wq")
            xT = pIN.tile([128, KQ * NQ], hdt, tag="xt")
            for k in range(KQ):
                nc.sync.dma_start(wq[:, k * G:(k + 1) * G],
                                  wq_d[k * 128:(k + 1) * 128, :])
                nc.sync.dma_start(xT[:, k * NQ:(k + 1) * NQ],
                                  xT_d[k * 128:(k + 1) * 128, :])
            wk = pIN.tile([128, KC * G], hdt, tag="wk")
            wv = pIN.tile([128, KC * G], hdt, tag="wv")
            ctxT = pIN.tile([128, KC * NKV], hdt, tag="ct")
            for k in range(KC):
                nc.sync.dma_start(wk[:, k * G:(k + 1) * G],
                                  wk_d[k * 128:(k + 1) * 128, :])
                nc.sync.dma_start(wv[:, k * G:(k + 1) * G],
                                  wv_d[k * 128:(k + 1) * 128, :])
                nc.sync.dma_start(ctxT[:, k * NKV:(k + 1) * NKV],
                                  ctxT_d[k * 128:(k + 1) * 128, :])
            wo = pIN.tile([128, MC * OUT_DIM], hdt, tag="wo")
            nc.sync.dma_start(
                wo[:].rearrange("p (c n) -> p c n", c=MC),
                wo_d.rearrange("(c p) n -> p c n", p=128))

            psAB = ctx.enter_context(
                tc.tile_pool(name="psAB", bufs=PSAB_BUFS, space="PSUM"))
            psS = ctx.enter_context(
                tc.tile_pool(name="psS", bufs=PSS_BUFS, space="PSUM"))
            psO = ctx.enter_context(
                tc.tile_pool(name="psO", bufs=PSO_BUFS, space="PSUM"))
            pP = ctx.enter_context(tc.tile_pool(name="pq", bufs=PQ_BUFS))
            pDen = ctx.enter_context(tc.tile_pool(name="den", bufs=2))

            def emit_A(m):
                for q in range(NQ // 512):
                    ps = psAB.tile([128, 512], f32, tag="ab")
                    for k in range(KQ):
                        nc.tensor.matmul(
                            ps[:],
                            wq[:, k * G + m * 128:k * G + (m + 1) * 128],
                            xT[:, k * NQ + q * 512:k * NQ + (q + 1) * 512],
                            start=(k == 0), stop=(k == KQ - 1))
                    nc.vector.tensor_copy(
                        QT[:, m * NQ + q * 512:m * NQ + (q + 1) * 512], ps[:])

            def emit_BK(m):
                for q in range(NKV // 512):
                    ps = psAB.tile([128, 512], f32, tag="ab")
                    for k in range(KC):
                        nc.tensor.matmul(
                            ps[:],
                            wk[:, k * G + m * 128:k * G + (m + 1) * 128],
                            ctxT[:, k * NKV + q * 512:k * NKV + (q + 1) * 512],
                            start=(k == 0), stop=(k == KC - 1))
                    nc.vector.tensor_copy(
                        KT[:, m * NKV + q * 512:m * NKV + (q + 1) * 512], ps[:])

            def emit_BV():
                nc.gpsimd.memset(
                    V[:].rearrange("p (c h e) -> p c h e",
                                   c=KVC, e=VW)[:, :, :, 64:65],
                    1.0)
                for kvc in range(KVC):
                    ps = psAB.tile([128, 512], f32, tag="ab")
                    for k in range(KC):
                        nc.tensor.matmul(
                            ps[:],
                            ctxT[:, k * NKV + kvc * 128:k * NKV + (kvc + 1) * 128],
                            wv[:, k * G:(k + 1) * G],
                            start=(k == 0), stop=(k == KC - 1))
                    nc.vector.tensor_copy(
                        V[:, kvc * HPC * VW:(kvc + 1) * HPC * VW]
                        .rearrange("p (h e) -> p h e", e=VW)[:, :, 0:64],
                        ps[:].rearrange("p (h e) -> p h e", e=64))

            def emit_C(m, bv=False):
                NG = KVC // cpt
                for q in range(nqblks):
                    qo = q * nqb
                    quads = ([], [])
                    pos = [None, None]

                    def av_chunks(kvg):
                        for hi in range(2):
                            if kvg == 0:
                                pos[hi] = psO.tile([VW, nqb], f32, tag="po",
                                                   name=f"po{hi}")
                            for j in range(cpt):
                                kvc = kvg * cpt + j
                                nc.tensor.matmul(
                                    pos[hi][:],
                                    V[:, kvc * HPC * VW + (2 * m + hi) * VW:
                                       kvc * HPC * VW + (2 * m + hi + 1) * VW],
                                    quads[hi][kvg][:, j * nqb:(j + 1) * nqb],
                                    start=(kvc == 0), stop=(kvc == KVC - 1))

                    def norm(hi):
                        po = pos[hi]
                        d = pDen.tile([1, nqb], f32, tag="d")
                        nc.vector.reciprocal(d[:], po[64:65, :])
                        R = pDen.tile([64, nqb], f32, tag="r")
                        nc.gpsimd.partition_broadcast(R[:], d[:])
                        nc.vector.tensor_mul(
                            OT[hi * 64:(hi + 1) * 64,
                               m * NQ + qo:m * NQ + qo + nqb],
                            po[0:64, :], R[:])

                    for kvg in range(NG):
                        s1 = psS.tile([128, cpt * nqb], f32, tag="s")
                        s2 = psS.tile([128, cpt * nqb], f32, tag="s")
                        for j in range(cpt):
                            kvc = kvg * cpt + j
                            ko = m * NKV + kvc * 128
                            nc.tensor.matmul(
                                s1[:, j * nqb:(j + 1) * nqb],
                                KT[0:64, ko:ko + 128],
                                QT[0:64, m * NQ + qo:m * NQ + qo + nqb],
                                start=True, stop=True, tile_position=(0, 0))
                            nc.tensor.matmul(
                                s2[:, j * nqb:(j + 1) * nqb],
                                KT[64:128, ko:ko + 128],
                                QT[64:128, m * NQ + qo:m * NQ + qo + nqb],
                                start=True, stop=True, tile_position=(64, 0))
                        for hi, s in ((0, s1), (1, s2)):
                            pquad = pP.tile([128, cpt * nqb], hdt, tag="pq")
                            nc.scalar.activation(pquad[:], s[:], EXP)
                            quads[hi].append(pquad)
                        if bv and q == 0 and kvg == 0:
                            emit_BV()
                        if kvg > 0:
                            av_chunks(kvg - 1)
                    av_chunks(NG - 1)
                    norm(0)
                    norm(1)

            def emit_D():
                with tc.tile_pool(name="outst", bufs=2) as pOut:
                    for mq in range(NQ // 128):
                        for n2 in range(OUT_DIM // 512):
                            ps = psAB.tile([128, 512], f32, tag="ab")
                            for c in range(MC):
                                nc.tensor.matmul(
                                    ps[:],
                                    OT[:, c * NQ + mq * 128:c * NQ + (mq + 1) * 128],
                                    wo[:, c * OUT_DIM + n2 * 512:
                                       c * OUT_DIM + (n2 + 1) * 512],
                                    start=(c == 0), stop=(c == MC - 1))
                            ob = pOut.tile([128, 512], f32)
                            nc.vector.tensor_copy(ob[:], ps[:])
                            nc.sync.dma_start(
                                out_d[mq * 128:(mq + 1) * 128,
                                      n2 * 512:(n2 + 1) * 512],
                                ob[:])

            for m in range(MC):
                emit_A(m)
                emit_BK(m)
                emit_C(m, bv=(m == 0))
            emit_D()

    with tile.TileContext(nc, trace_sim=False) as tc:
        if reps == 1:
            _emit(tc)
        else:
            with tc.For_i(0, reps, 1):
                _emit(tc)

    nc.compile()
    return nc


def get_program(reps=1, mmdt="bfloat16", **kw):
    key = ("prog", reps, mmdt)
    if key not in _CACHE:
        _CACHE[key] = _build_program(reps=reps, mmdt=mmdt)
    return _CACHE[key]


def make_in_maps(x, context, Wq, Wk, Wv, Wo, mmdt="bfloat16"):
    import ml_dtypes
    hdt = np.float32 if mmdt == "float32r" else np.dtype(ml_dtypes.bfloat16)
    x = np.asarray(x, dtype=np.float32)
    context = np.asarray(context, dtype=np.float32)
    Wq = np.asarray(Wq, dtype=np.float32)
    Wk = np.asarray(Wk, dtype=np.float32)
    Wv = np.asarray(Wv, dtype=np.float32)
    Wo = np.asarray(Wo, dtype=np.float32)
    xT = [np.ascontiguousarray(x[b].T).astype(hdt) for b in range(B)]
    ctxT = [np.ascontiguousarray(context[b].T).astype(hdt) for b in range(B)]
    wq = [(np.ascontiguousarray(Wq[:, g * G:(g + 1) * G]) * np.float32(SCALE))
          .astype(hdt) for g in range(2)]
    wk = [np.ascontiguousarray(Wk[:, g * G:(g + 1) * G]).astype(hdt) for g in range(2)]
    wv = [np.ascontiguousarray(Wv[:, g * G:(g + 1) * G]).astype(hdt) for g in range(2)]
    wo = [np.ascontiguousarray(Wo[g * G:(g + 1) * G, :]).astype(hdt) for g in range(2)]
    in_maps = []
    for c in range(8):
        b, g = c // 2, c % 2
        in_maps.append({"xT": xT[b], "ctxT": ctxT[b], "wq": wq[g],
                        "wk": wk[g], "wv": wv[g], "wo": wo[g]})
    return in_maps


def run_device(nc, in_maps):
    return bass_utils.run_bass_kernel_spmd(nc, in_maps, core_ids=list(range(8)))


def kernel(x, context, Wq, Wk, Wv, Wo, bo, mmdt="bfloat16"):
    nc = get_program(mmdt=mmdt)
    in_maps = make_in_maps(x, context, Wq, Wk, Wv, Wo, mmdt=mmdt)
    res = run_device(nc, in_maps)
    bo = np.asarray(bo, dtype=np.float32)
    out = np.empty((B, NQ, OUT_DIM), dtype=np.float32)
    for b in range(B):
        out[b] = res.results[2 * b]["out"] + res.results[2 * b + 1]["out"] + bo
    return out


# revision 3
# speedup vs baseline: 1.2425x; 1.2044x over previous
"""Cross-attention Trainium2 Bass kernel (optimized).

Sharding: 8 cores = 4 batches x 2 head-groups (8 heads each); each core
computes a partial output [2048, 1024] through its half of Wo; the host
sums the two partials per batch and adds the bias.  Matmul operands bf16.

Structure (single TileContext, scheduler-visible cross-phase overlap):
- Q/K projections emitted per head-pair m, interleaved with the attention
  phase C(m), so the tensor engine fills softmax-exp waits (ACT engine's
  33.5M exps/core are the serial floor) with projection matmuls.
- C(m) is software-pipelined per 512-wide query block: per kv-group the
  row-packed S^T matmul pair (2 heads via tile_position 0/64, K=64), the
  two PSUM->SBUF bf16 exps, then the attn@V chunk matmuls of the previous
  kv-group.  Per-head 65th V column = 1.0 makes attn@V also produce the
  softmax denominators; normalization broadcasts 1/den across partitions
  with a PE outer product into the unused half of the attn@V PSUM bank.
- V projection emitted under C(0)'s first exp group; input DMAs are
  chunk-split so the first projection starts after one 512KB chunk.
"""
import sys

sys.path.insert(0, "/opt/trn_rl_repo")

import numpy as np

import concourse.bass as bass  # noqa: F401
import concourse.tile as tile
from concourse import bacc, mybir
from concourse import bass_utils

B = 4
NQ = 2048
NKV = 2048
IN_DIM = 1024
CTX_DIM = 768
N_HEADS = 16
HEAD_DIM = 64
G = 512
HPC = 8
OUT_DIM = 1024
SCALE = HEAD_DIM ** -0.5
KVC = NKV // 128
VW = HEAD_DIM + 1

_CACHE = {}


def _build_program(reps=1, mmdt="bfloat16", nqb=512, pack_s=True,
                   interleave=True, rep_c=1, rep_ab=1, rep_d=1,
                   pq_extra=6, pss_bufs=2, pso_bufs=2, psab_bufs=2,
                   c_parts="full", pipec=True, bv_mid=True, c_parts_all=None,
                   norm_pe=True, den_bufs=3, qouter=False):
    f32 = mybir.dt.float32
    hdt = getattr(mybir.dt, mmdt)
    EXP = mybir.ActivationFunctionType.Exp
    nqblks = NQ // nqb
    cpt = 1024 // nqb            # kv chunks per S-psum tile (tile = 2 banks)
    KQ = IN_DIM // 128
    KC = CTX_DIM // 128
    MC = G // 128

    nc = bacc.Bacc("TRN2", target_bir_lowering=False, debug=False,
                   enable_asserts=False, num_devices=8)
    xT_d = nc.dram_tensor("xT", [IN_DIM, NQ], hdt, kind="ExternalInput").ap()
    ctxT_d = nc.dram_tensor("ctxT", [CTX_DIM, NKV], hdt, kind="ExternalInput").ap()
    wq_d = nc.dram_tensor("wq", [IN_DIM, G], hdt, kind="ExternalInput").ap()
    wk_d = nc.dram_tensor("wk", [CTX_DIM, G], hdt, kind="ExternalInput").ap()
    wv_d = nc.dram_tensor("wv", [CTX_DIM, G], hdt, kind="ExternalInput").ap()
    wo_d = nc.dram_tensor("wo", [G, OUT_DIM], hdt, kind="ExternalInput").ap()
    out_d = nc.dram_tensor("out", [NQ, OUT_DIM], f32, kind="ExternalOutput").ap()

    from contextlib import ExitStack

    def _emit(tc):
        with ExitStack() as ctx:
            pQT = ctx.enter_context(tc.tile_pool(name="qt", bufs=1))
            pKT = ctx.enter_context(tc.tile_pool(name="kt", bufs=1))
            pV = ctx.enter_context(tc.tile_pool(name="vv", bufs=1))
            pOT = ctx.enter_context(tc.tile_pool(name="ot", bufs=1))
            pIN = ctx.enter_context(tc.tile_pool(name="ins", bufs=1))
            QT = pQT.tile([128, MC * NQ], hdt)
            KT = pKT.tile([128, MC * NKV], hdt)
            V = pV.tile([128, KVC * HPC * VW], hdt)
            OT = pOT.tile([128, MC * NQ], hdt)
            if c_parts_all in ("se", "sea"):
                # reduced probe variants never write OT fully; keep the tile
                # allocator happy (D reads garbage, timing-only)
                nc.gpsimd.memset(OT[:].bitcast(mybir.dt.float32), 0.0)
            ones_sb = None
            if norm_pe:
                pONE = ctx.enter_context(tc.tile_pool(name="one", bufs=1))
                ones_sb = pONE.tile([1, 64], hdt)
                nc.gpsimd.memset(ones_sb[:], 1.0)

            # --- input DMAs (all early; weights+x chunk-split so the first
            # projection matmuls start as soon as chunk 0 lands) -----------
            wq = pIN.tile([128, KQ * G], hdt, tag="wq")
            xT = pIN.tile([128, KQ * NQ], hdt, tag="xt")
            for k in range(KQ):
                nc.sync.dma_start(
                    wq[:, k * G:(k + 1) * G],
                    wq_d[k * 128:(k + 1) * 128, :])
                nc.sync.dma_start(
                    xT[:, k * NQ:(k + 1) * NQ],
                    xT_d[k * 128:(k + 1) * 128, :])
            wk = pIN.tile([128, KC * G], hdt, tag="wk")
            wv = pIN.tile([128, KC * G], hdt, tag="wv")
            ctxT = pIN.tile([128, KC * NKV], hdt, tag="ct")
            for k in range(KC):
                nc.sync.dma_start(
                    wk[:, k * G:(k + 1) * G],
                    wk_d[k * 128:(k + 1) * 128, :])
                nc.sync.dma_start(
                    wv[:, k * G:(k + 1) * G],
                    wv_d[k * 128:(k + 1) * 128, :])
                nc.sync.dma_start(
                    ctxT[:, k * NKV:(k + 1) * NKV],
                    ctxT_d[k * 128:(k + 1) * 128, :])
            wo = pIN.tile([128, MC * OUT_DIM], hdt, tag="wo")
            nc.sync.dma_start(
                wo[:].rearrange("p (c n) -> p c n", c=MC),
                wo_d.rearrange("(c p) n -> p c n", p=128))

            psAB = ctx.enter_context(
                tc.tile_pool(name="psAB", bufs=psab_bufs, space="PSUM"))
            psS = ctx.enter_context(
                tc.tile_pool(name="psS", bufs=pss_bufs, space="PSUM"))
            psO = ctx.enter_context(
                tc.tile_pool(name="psO", bufs=pso_bufs, space="PSUM"))
            nquads = 2 * (KVC // cpt)
            pP = ctx.enter_context(
                tc.tile_pool(name="pq", bufs=nquads + pq_extra))
            pDen = ctx.enter_context(tc.tile_pool(name="den", bufs=den_bufs))

            def emit_A(m):
                for q in range(NQ // 512):
                    ps = psAB.tile([128, 512], f32, tag="ab")
                    for k in range(KQ):
                        nc.tensor.matmul(
                            ps[:],
                            wq[:, k * G + m * 128:k * G + (m + 1) * 128],
                            xT[:, k * NQ + q * 512:k * NQ + (q + 1) * 512],
                            start=(k == 0), stop=(k == KQ - 1))
                    nc.vector.tensor_copy(
                        QT[:, m * NQ + q * 512:m * NQ + (q + 1) * 512], ps[:])

            def emit_BK(m):
                for q in range(NKV // 512):
                    ps = psAB.tile([128, 512], f32, tag="ab")
                    for k in range(KC):
                        nc.tensor.matmul(
                            ps[:],
                            wk[:, k * G + m * 128:k * G + (m + 1) * 128],
                            ctxT[:, k * NKV + q * 512:k * NKV + (q + 1) * 512],
                            start=(k == 0), stop=(k == KC - 1))
                    nc.vector.tensor_copy(
                        KT[:, m * NKV + q * 512:m * NKV + (q + 1) * 512], ps[:])

            def emit_BV():
                ones_view = V[:].bitcast(f32) if mmdt == "float32r" else V[:]
                nc.gpsimd.memset(
                    ones_view
                    .rearrange("p (c h e) -> p c h e", c=KVC, e=VW)[:, :, :, 64:65],
                    1.0)
                for kvc in range(KVC):
                    ps = psAB.tile([128, 512], f32, tag="ab")
                    for k in range(KC):
                        nc.tensor.matmul(
                            ps[:],
                            ctxT[:, k * NKV + kvc * 128:k * NKV + (kvc + 1) * 128],
                            wv[:, k * G:(k + 1) * G],
                            start=(k == 0), stop=(k == KC - 1))
                    nc.vector.tensor_copy(
                        V[:, kvc * HPC * VW:(kvc + 1) * HPC * VW]
                        .rearrange("p (h e) -> p h e", e=VW)[:, :, 0:64],
                        ps[:].rearrange("p (h e) -> p h e", e=64))

            def emit_C_pipe(m, bv=False, qlist=None):
                """Software-pipelined C: per kv-group, emit S pair, exps,
                then the AV chunk-matmuls of the PREVIOUS kv-group, so the
                in-order PE stream interleaves S and AV at ACT pace.
                bv=True: emit the V projection after the first S/exp group
                (it then fills PE idle under C(0)'s first exps)."""
                NG = KVC // cpt
                for q in (range(nqblks) if qlist is None else qlist):
                    qo = q * nqb
                    quads = ([], [])
                    pos = [None, None]

                    def av_chunks(kvg):
                        for hi in range(2):
                            if kvg == 0:
                                pos[hi] = psO.tile(
                                    [128 if norm_pe else VW, nqb], f32,
                                    tag="po", name=f"po{hi}")
                            for j in range(cpt):
                                kvc = kvg * cpt + j
                                nc.tensor.matmul(
                                    pos[hi][0:VW, :],
                                    V[:, kvc * HPC * VW + (2 * m + hi) * VW:
                                       kvc * HPC * VW + (2 * m + hi + 1) * VW],
                                    quads[hi][kvg][:, j * nqb:(j + 1) * nqb],
                                    start=(kvc == 0), stop=(kvc == KVC - 1))

                    def norm(hi):
                        po = pos[hi]
                        d = pDen.tile([1, nqb], hdt if norm_pe else f32,
                                      tag="d")
                        if norm_pe:
                            with nc.allow_low_precision(
                                    reason="1/den broadcast operand; bf16 "
                                           "rounding is within the rel-err "
                                           "budget"):
                                nc.vector.reciprocal(d[:], po[64:65, :])
                        else:
                            nc.vector.reciprocal(d[:], po[64:65, :])
                        R = pDen.tile([64, nqb], f32, tag="r")
                        if norm_pe:
                            # broadcast 1/den across partitions via a PE
                            # outer product into the unused half of po's
                            # bank, then stage to SBUF (DVE reads at most
                            # one PSUM operand)
                            nc.tensor.matmul(po[64:128, :], ones_sb[:], d[:],
                                             start=True, stop=True)
                            nc.vector.tensor_copy(R[:], po[64:128, :])
                        else:
                            nc.gpsimd.partition_broadcast(R[:], d[:])
                        nc.vector.tensor_mul(
                            OT[hi * 64:(hi + 1) * 64,
                               m * NQ + qo:m * NQ + qo + nqb],
                            po[0:64, :], R[:])

                    for kvg in range(NG):
                        s1 = psS.tile([128, cpt * nqb], f32, tag="s")
                        s2 = psS.tile([128, cpt * nqb], f32, tag="s")
                        for j in range(cpt):
                            kvc = kvg * cpt + j
                            ko = m * NKV + kvc * 128
                            kw = dict(start=True, stop=True)
                            if pack_s:
                                kw1 = dict(tile_position=(0, 0), **kw)
                                kw2 = dict(tile_position=(64, 0), **kw)
                            else:
                                kw1 = kw2 = kw
                            nc.tensor.matmul(
                                s1[:, j * nqb:(j + 1) * nqb],
                                KT[0:64, ko:ko + 128],
                                QT[0:64, m * NQ + qo:m * NQ + qo + nqb], **kw1)
                            nc.tensor.matmul(
                                s2[:, j * nqb:(j + 1) * nqb],
                                KT[64:128, ko:ko + 128],
                                QT[64:128, m * NQ + qo:m * NQ + qo + nqb], **kw2)
                        for hi, s in ((0, s1), (1, s2)):
                            pquad = pP.tile([128, cpt * nqb], hdt, tag="pq")
                            nc.scalar.activation(pquad[:], s[:], EXP)
                            quads[hi].append(pquad)
                        if bv and q == 0 and kvg == 0:
                            emit_BV()
                        if kvg > 0:
                            av_chunks(kvg - 1)
                    av_chunks(NG - 1)
                    norm(0)
                    norm(1)

            def emit_C(m, parts="full"):
                do_exp = parts in ("se", "sea", "full")
                do_av = parts in ("sea", "full", "noexp")
                do_norm = parts in ("full", "noexp")
                for q in range(nqblks):
                    qo = q * nqb
                    quads = ([], [])
                    for kvg in range(KVC // cpt):
                        s1 = psS.tile([128, cpt * nqb], f32, tag="s")
                        s2 = psS.tile([128, cpt * nqb], f32, tag="s")
                        for j in range(cpt):
                            kvc = kvg * cpt + j
                            ko = m * NKV + kvc * 128
                            kw = dict(start=True, stop=True)
                            if pack_s:
                                kw1 = dict(tile_position=(0, 0), **kw)
                                kw2 = dict(tile_position=(64, 0), **kw)
                            else:
                                kw1 = kw2 = kw
                            nc.tensor.matmul(
                                s1[:, j * nqb:(j + 1) * nqb],
                                KT[0:64, ko:ko + 128],
                                QT[0:64, m * NQ + qo:m * NQ + qo + nqb], **kw1)
                            nc.tensor.matmul(
                                s2[:, j * nqb:(j + 1) * nqb],
                                KT[64:128, ko:ko + 128],
                                QT[64:128, m * NQ + qo:m * NQ + qo + nqb], **kw2)
                        if not (do_exp or do_av):
                            continue
                        for hi, s in ((0, s1), (1, s2)):
                            pquad = pP.tile([128, cpt * nqb], hdt, tag="pq")
                            if do_exp:
                                nc.scalar.activation(pquad[:], s[:], EXP)
                            quads[hi].append(pquad)
                    if not do_av:
                        continue
                    for hi in range(2):
                        h = 2 * m + hi
                        po = psO.tile([VW, nqb], f32)
                        for kvc in range(KVC):
                            nc.tensor.matmul(
                                po[:],
                                V[:, kvc * HPC * VW + h * VW:
                                   kvc * HPC * VW + (h + 1) * VW],
                                quads[hi][kvc // cpt][:, (kvc % cpt) * nqb:
                                                      (kvc % cpt + 1) * nqb],
                                start=(kvc == 0), stop=(kvc == KVC - 1))
                        if not do_norm:
                            continue
                        d = pDen.tile([1, nqb], f32, tag="d")
                        nc.vector.reciprocal(d[:], po[64:65, :])
                        R = pDen.tile([64, nqb], f32, tag="r")
                        nc.gpsimd.partition_broadcast(R[:], d[:])
                        nc.vector.tensor_mul(
                            OT[hi * 64:(hi + 1) * 64, m * NQ + qo:m * NQ + qo + nqb],
                            po[0:64, :], R[:])

            def emit_D_range(pOutp, mqs):
                for mq in mqs:
                    for n2 in range(OUT_DIM // 512):
                        ps = psAB.tile([128, 512], f32, tag="ab")
                        for c in range(MC):
                            nc.tensor.matmul(
                                ps[:],
                                OT[:, c * NQ + mq * 128:c * NQ + (mq + 1) * 128],
                                wo[:, c * OUT_DIM + n2 * 512:c * OUT_DIM + (n2 + 1) * 512],
                                start=(c == 0), stop=(c == MC - 1))
                        ob = pOutp.tile([128, 512], f32, tag="ob", name="ob")
                        nc.vector.tensor_copy(ob[:], ps[:])
                        nc.sync.dma_start(
                            out_d[mq * 128:(mq + 1) * 128,
                                  n2 * 512:(n2 + 1) * 512],
                            ob[:])

            def emit_D():
                pOut = tc.tile_pool(name="outst", bufs=2)
                with pOut as pOutp:
                    emit_D_range(pOutp, range(NQ // 128))

            if qouter:
                # q-outer: all head-pairs per query block, so phase D for
                # block q-1 is ready while block q computes and overlaps C.
                pOutQ = ctx.enter_context(tc.tile_pool(name="outst", bufs=2))
                mq_per_q = nqb // 128
                for m in range(MC):
                    emit_A(m)
                    emit_BK(m)
                    if m == 0 and not bv_mid:
                        emit_BV()
                    emit_C_pipe(m, bv=(bv_mid and m == 0), qlist=[0])
                for q in range(1, nqblks):
                    for m in range(MC):
                        emit_C_pipe(m, qlist=[q])
                    emit_D_range(pOutQ,
                                 range((q - 1) * mq_per_q, q * mq_per_q))
                emit_D_range(pOutQ,
                             range((nqblks - 1) * mq_per_q, nqblks * mq_per_q))
                return
            for r in range(rep_ab):
                if interleave:
                    for m in range(MC):
                        emit_A(m)
                        emit_BK(m)
                        if m == 0 and not bv_mid:
                            emit_BV()
                        if r == rep_ab - 1:
                            for rc in range(rep_c):
                                if pipec and rc == rep_c - 1:
                                    emit_C_pipe(m, bv=(bv_mid and m == 0
                                                       and rc == rep_c - 1))
                                else:
                                    emit_C(m, parts=(c_parts_all or
                                                     (c_parts if rc < rep_c - 1
                                                      else "full")))
                else:
                    for m in range(MC):
                        emit_A(m)
                    for m in range(MC):
                        emit_BK(m)
                    emit_BV()
                    if r == rep_ab - 1:
                        for rc in range(rep_c):
                            for m in range(MC):
                                if pipec and rc == rep_c - 1:
                                    emit_C_pipe(m)
                                else:
                                    emit_C(m, parts=(c_parts_all or
                                                     (c_parts if rc < rep_c - 1
                                                      else "full")))
            for rd in range(rep_d):
                emit_D()

    with tile.TileContext(nc, trace_sim=False) as tc:
        if reps == 1:
            _emit(tc)
        else:
            with tc.For_i(0, reps, 1):
                _emit(tc)

    nc.compile()
    return nc


def get_program(reps=1, mmdt="bfloat16", **kw):
    key = ("prog", reps, mmdt, tuple(sorted(kw.items())))
    if key not in _CACHE:
        _CACHE[key] = _build_program(reps=reps, mmdt=mmdt, **kw)
    return _CACHE[key]


def make_in_maps(x, context, Wq, Wk, Wv, Wo, mmdt="bfloat16"):
    import ml_dtypes
    hdt = np.float32 if mmdt == "float32r" else np.dtype(ml_dtypes.bfloat16)
    x = np.asarray(x, dtype=np.float32)
    context = np.asarray(context, dtype=np.float32)
    Wq = np.asarray(Wq, dtype=np.float32)
    Wk = np.asarray(Wk, dtype=np.float32)
    Wv = np.asarray(Wv, dtype=np.float32)
    Wo = np.asarray(Wo, dtype=np.float32)
    xT = [np.ascontiguousarray(x[b].T).astype(hdt) for b in range(B)]
    ctxT = [np.ascontiguousarray(context[b].T).astype(hdt) for b in range(B)]
    wq = [(np.ascontiguousarray(Wq[:, g * G:(g + 1) * G]) * np.float32(SCALE))
          .astype(hdt) for g in range(2)]
    wk = [np.ascontiguousarray(Wk[:, g * G:(g + 1) * G]).astype(hdt) for g in range(2)]
    wv = [np.ascontiguousarray(Wv[:, g * G:(g + 1) * G]).astype(hdt) for g in range(2)]
    wo = [np.ascontiguousarray(Wo[g * G:(g + 1) * G, :]).astype(hdt) for g in range(2)]
    in_maps = []
    for c in range(8):
        b, g = c // 2, c % 2
        in_maps.append({"xT": xT[b], "ctxT": ctxT[b], "wq": wq[g],
                        "wk": wk[g], "wv": wv[g], "wo": wo[g]})
    return in_maps


def kernel(x, context, Wq, Wk, Wv, Wo, bo, mmdt="bfloat16", **kw):
    nc = get_program(mmdt=mmdt, **kw)
    in_maps = make_in_maps(x, context, Wq, Wk, Wv, Wo, mmdt=mmdt)
    res = bass_utils.run_bass_kernel_spmd(nc, in_maps, core_ids=list(range(8)))
    bo = np.asarray(bo, dtype=np.float32)
    out = np.empty((B, NQ, OUT_DIM), dtype=np.float32)
    for b in range(B):
        out[b] = res.results[2 * b]["out"] + res.results[2 * b + 1]["out"] + bo
    return out


# revision 4
# speedup vs baseline: 1.3946x; 1.1225x over previous
"""Cross-attention Trainium2 Bass kernel (optimized).

Sharding: 8 cores = 4 batches x 2 head-groups (8 heads each); each core
computes a partial output [2048, 1024] through its half of Wo; the host
sums the two partials per batch and adds the bias.  Matmul operands bf16.

Structure (single TileContext, scheduler-visible cross-phase overlap):
- Q/K projections emitted per head-pair m, interleaved with the attention
  phase C(m), so the tensor engine fills softmax-exp waits (ACT engine's
  33.5M exps/core are the serial floor) with projection matmuls.
- C(m) is software-pipelined per 512-wide query block: per kv-group the
  row-packed S^T matmul pair (2 heads via tile_position 0/64, K=64), the
  two PSUM->SBUF bf16 exps, then the attn@V chunk matmuls of the previous
  kv-group.  Per-head 65th V column = 1.0 makes attn@V also produce the
  softmax denominators.
- Normalization stages the attn@V PSUM accumulator to SBUF with one DVE
  copy so the PSUM slot recycles in ~0.6us instead of waiting the full
  reciprocal/broadcast/multiply chain (measured -95us wall), then
  normalizes out of SBUF off the critical path.
- V projection emitted under C(0)'s first exp group; input DMAs are
  chunk-split so the first projection starts after one 512KB chunk.
"""
import sys

sys.path.insert(0, "/opt/trn_rl_repo")

import numpy as np

import concourse.bass as bass  # noqa: F401
import concourse.tile as tile
from concourse import bacc, mybir
from concourse import bass_utils

B = 4
NQ = 2048
NKV = 2048
IN_DIM = 1024
CTX_DIM = 768
N_HEADS = 16
HEAD_DIM = 64
G = 512
HPC = 8
OUT_DIM = 1024
SCALE = HEAD_DIM ** -0.5
KVC = NKV // 128
VW = HEAD_DIM + 1

_CACHE = {}


def _build_program(reps=1, mmdt="bfloat16", nqb=512, pack_s=True,
                   interleave=True, rep_c=1, rep_ab=1, rep_d=1,
                   pq_extra=5, pss_bufs=2, pso_bufs=2, psab_bufs=2,
                   c_parts="full", pipec=True, bv_mid=True, c_parts_all=None,
                   norm_pe=False, den_bufs=2, qouter=False, mixexp=False,
                   stage_po=True):
    f32 = mybir.dt.float32
    hdt = getattr(mybir.dt, mmdt)
    EXP = mybir.ActivationFunctionType.Exp
    nqblks = NQ // nqb
    cpt = 1024 // nqb            # kv chunks per S-psum tile (tile = 2 banks)
    KQ = IN_DIM // 128
    KC = CTX_DIM // 128
    MC = G // 128

    nc = bacc.Bacc("TRN2", target_bir_lowering=False, debug=False,
                   enable_asserts=False, num_devices=8)
    xT_d = nc.dram_tensor("xT", [IN_DIM, NQ], hdt, kind="ExternalInput").ap()
    ctxT_d = nc.dram_tensor("ctxT", [CTX_DIM, NKV], hdt, kind="ExternalInput").ap()
    wq_d = nc.dram_tensor("wq", [IN_DIM, G], hdt, kind="ExternalInput").ap()
    wk_d = nc.dram_tensor("wk", [CTX_DIM, G], hdt, kind="ExternalInput").ap()
    wv_d = nc.dram_tensor("wv", [CTX_DIM, G], hdt, kind="ExternalInput").ap()
    wo_d = nc.dram_tensor("wo", [G, OUT_DIM], hdt, kind="ExternalInput").ap()
    out_d = nc.dram_tensor("out", [NQ, OUT_DIM], f32, kind="ExternalOutput").ap()

    from contextlib import ExitStack

    def _emit(tc):
        with ExitStack() as ctx:
            pQT = ctx.enter_context(tc.tile_pool(name="qt", bufs=1))
            pKT = ctx.enter_context(tc.tile_pool(name="kt", bufs=1))
            pV = ctx.enter_context(tc.tile_pool(name="vv", bufs=1))
            pOT = ctx.enter_context(tc.tile_pool(name="ot", bufs=1))
            pIN = ctx.enter_context(tc.tile_pool(name="ins", bufs=1))
            QT = pQT.tile([128, MC * NQ], hdt)
            KT = pKT.tile([128, MC * NKV], hdt)
            V = pV.tile([128, KVC * HPC * VW], hdt)
            OT = pOT.tile([128, MC * NQ], hdt)
            if c_parts_all in ("se", "sea"):
                # reduced probe variants never write OT fully; keep the tile
                # allocator happy (D reads garbage, timing-only)
                nc.gpsimd.memset(OT[:].bitcast(mybir.dt.float32), 0.0)
            ones_sb = None
            if norm_pe:
                pONE = ctx.enter_context(tc.tile_pool(name="one", bufs=1))
                ones_sb = pONE.tile([1, 64], hdt)
                nc.gpsimd.memset(ones_sb[:], 1.0)

            # --- input DMAs (all early; weights+x chunk-split so the first
            # projection matmuls start as soon as chunk 0 lands) -----------
            wq = pIN.tile([128, KQ * G], hdt, tag="wq")
            xT = pIN.tile([128, KQ * NQ], hdt, tag="xt")
            for k in range(KQ):
                nc.sync.dma_start(
                    wq[:, k * G:(k + 1) * G],
                    wq_d[k * 128:(k + 1) * 128, :])
                nc.sync.dma_start(
                    xT[:, k * NQ:(k + 1) * NQ],
                    xT_d[k * 128:(k + 1) * 128, :])
            wk = pIN.tile([128, KC * G], hdt, tag="wk")
            wv = pIN.tile([128, KC * G], hdt, tag="wv")
            ctxT = pIN.tile([128, KC * NKV], hdt, tag="ct")
            for k in range(KC):
                nc.sync.dma_start(
                    wk[:, k * G:(k + 1) * G],
                    wk_d[k * 128:(k + 1) * 128, :])
                nc.sync.dma_start(
                    wv[:, k * G:(k + 1) * G],
                    wv_d[k * 128:(k + 1) * 128, :])
                nc.sync.dma_start(
                    ctxT[:, k * NKV:(k + 1) * NKV],
                    ctxT_d[k * 128:(k + 1) * 128, :])
            wo = pIN.tile([128, MC * OUT_DIM], hdt, tag="wo")
            nc.sync.dma_start(
                wo[:].rearrange("p (c n) -> p c n", c=MC),
                wo_d.rearrange("(c p) n -> p c n", p=128))

            psAB = ctx.enter_context(
                tc.tile_pool(name="psAB", bufs=psab_bufs, space="PSUM"))
            psS = ctx.enter_context(
                tc.tile_pool(name="psS", bufs=pss_bufs, space="PSUM"))
            psBig = None
            if mixexp:
                psBig = ctx.enter_context(
                    tc.tile_pool(name="psBig", bufs=1, space="PSUM"))
            psO = ctx.enter_context(
                tc.tile_pool(name="psO", bufs=pso_bufs, space="PSUM"))
            nquads = 2 * (KVC // cpt)
            pP = ctx.enter_context(
                tc.tile_pool(name="pq", bufs=nquads + pq_extra))
            pDen = ctx.enter_context(tc.tile_pool(name="den", bufs=den_bufs))

            def emit_A(m):
                for q in range(NQ // 512):
                    ps = psAB.tile([128, 512], f32, tag="ab")
                    for k in range(KQ):
                        nc.tensor.matmul(
                            ps[:],
                            wq[:, k * G + m * 128:k * G + (m + 1) * 128],
                            xT[:, k * NQ + q * 512:k * NQ + (q + 1) * 512],
                            start=(k == 0), stop=(k == KQ - 1))
                    nc.vector.tensor_copy(
                        QT[:, m * NQ + q * 512:m * NQ + (q + 1) * 512], ps[:])

            def emit_BK(m):
                for q in range(NKV // 512):
                    ps = psAB.tile([128, 512], f32, tag="ab")
                    for k in range(KC):
                        nc.tensor.matmul(
                            ps[:],
                            wk[:, k * G + m * 128:k * G + (m + 1) * 128],
                            ctxT[:, k * NKV + q * 512:k * NKV + (q + 1) * 512],
                            start=(k == 0), stop=(k == KC - 1))
                    nc.vector.tensor_copy(
                        KT[:, m * NKV + q * 512:m * NKV + (q + 1) * 512], ps[:])

            def emit_BV():
                ones_view = V[:].bitcast(f32) if mmdt == "float32r" else V[:]
                nc.gpsimd.memset(
                    ones_view
                    .rearrange("p (c h e) -> p c h e", c=KVC, e=VW)[:, :, :, 64:65],
                    1.0)
                for kvc in range(KVC):
                    ps = psAB.tile([128, 512], f32, tag="ab")
                    for k in range(KC):
                        nc.tensor.matmul(
                            ps[:],
                            ctxT[:, k * NKV + kvc * 128:k * NKV + (kvc + 1) * 128],
                            wv[:, k * G:(k + 1) * G],
                            start=(k == 0), stop=(k == KC - 1))
                    nc.vector.tensor_copy(
                        V[:, kvc * HPC * VW:(kvc + 1) * HPC * VW]
                        .rearrange("p (h e) -> p h e", e=VW)[:, :, 0:64],
                        ps[:].rearrange("p (h e) -> p h e", e=64))

            def emit_C_mix(m, bv=False, qlist=None):
                """Like emit_C_pipe but with mixed-size combined S tiles:
                alternating [128,2048] (2 chunks x 2 heads, one N=2048 exp)
                and [128,1024] (1 chunk x 2 heads, one N=1024 exp) tiles,
                halving ACT per-instruction overhead."""
                sizes = [2, 1] * 5 + [1]          # chunk counts, sum = 16
                for q in (range(nqblks) if qlist is None else qlist):
                    qo = q * nqb
                    pos = [None, None]

                    def av_group(grp):
                        c0, sz, pq = grp
                        for hi in range(2):
                            if c0 == 0:
                                pos[hi] = psO.tile(
                                    [128 if norm_pe else VW, nqb], f32,
                                    tag="po", name=f"po{hi}")
                            for j in range(sz):
                                kvc = c0 + j
                                nc.tensor.matmul(
                                    pos[hi][0:VW, :],
                                    V[:, kvc * HPC * VW + (2 * m + hi) * VW:
                                       kvc * HPC * VW + (2 * m + hi + 1) * VW],
                                    pq[:, (hi * sz + j) * nqb:
                                       (hi * sz + j + 1) * nqb],
                                    start=(kvc == 0), stop=(kvc == KVC - 1))

                    def norm(hi):
                        po = pos[hi]
                        d = pDen.tile([1, nqb], hdt if norm_pe else f32,
                                      tag="d", name="d")
                        if norm_pe:
                            with nc.allow_low_precision(
                                    reason="1/den broadcast operand; bf16 "
                                           "rounding within rel-err budget"):
                                nc.vector.reciprocal(d[:], po[64:65, :])
                        else:
                            nc.vector.reciprocal(d[:], po[64:65, :])
                        R = pDen.tile([64, nqb], f32, tag="r", name="R")
                        if norm_pe:
                            nc.tensor.matmul(po[64:128, :], ones_sb[:], d[:],
                                             start=True, stop=True)
                            nc.vector.tensor_copy(R[:], po[64:128, :])
                        else:
                            nc.gpsimd.partition_broadcast(R[:], d[:])
                        nc.vector.tensor_mul(
                            OT[hi * 64:(hi + 1) * 64,
                               m * NQ + qo:m * NQ + qo + nqb],
                            po[0:64, :], R[:])

                    prev = None
                    c0 = 0
                    for gi, sz in enumerate(sizes):
                        if sz == 2:
                            stile = psBig.tile([128, 4 * nqb], f32, tag="sb",
                                               name="sbig")
                        else:
                            stile = psS.tile([128, 2 * nqb], f32, tag="s",
                                             name="ssml")
                        for j in range(sz):
                            kvc = c0 + j
                            ko = m * NKV + kvc * 128
                            nc.tensor.matmul(
                                stile[:, j * nqb:(j + 1) * nqb],
                                KT[0:64, ko:ko + 128],
                                QT[0:64, m * NQ + qo:m * NQ + qo + nqb],
                                start=True, stop=True, tile_position=(0, 0))
                            nc.tensor.matmul(
                                stile[:, (sz + j) * nqb:(sz + j + 1) * nqb],
                                KT[64:128, ko:ko + 128],
                                QT[64:128, m * NQ + qo:m * NQ + qo + nqb],
                                start=True, stop=True, tile_position=(64, 0))
                        pq = pP.tile([128, 2 * sz * nqb], hdt,
                                     tag=("pqb" if sz == 2 else "pqs"),
                                     bufs=(6 if sz == 2 else 8), name="pq")
                        nc.scalar.activation(pq[:], stile[:], EXP)
                        if bv and q == 0 and gi == 0:
                            emit_BV()
                        if prev is not None:
                            av_group(prev)
                        prev = (c0, sz, pq)
                        c0 += sz
                    av_group(prev)
                    norm(0)
                    norm(1)

            def emit_C_pipe(m, bv=False, qlist=None):
                """Software-pipelined C: per kv-group, emit S pair, exps,
                then the AV chunk-matmuls of the PREVIOUS kv-group, so the
                in-order PE stream interleaves S and AV at ACT pace.
                bv=True: emit the V projection after the first S/exp group
                (it then fills PE idle under C(0)'s first exps)."""
                NG = KVC // cpt
                for q in (range(nqblks) if qlist is None else qlist):
                    qo = q * nqb
                    quads = ([], [])
                    pos = [None, None]

                    def av_chunks(kvg):
                        for hi in range(2):
                            if kvg == 0:
                                pos[hi] = psO.tile(
                                    [128 if norm_pe else VW, nqb], f32,
                                    tag="po", name=f"po{hi}")
                            for j in range(cpt):
                                kvc = kvg * cpt + j
                                nc.tensor.matmul(
                                    pos[hi][0:VW, :],
                                    V[:, kvc * HPC * VW + (2 * m + hi) * VW:
                                       kvc * HPC * VW + (2 * m + hi + 1) * VW],
                                    quads[hi][kvg][:, j * nqb:(j + 1) * nqb],
                                    start=(kvc == 0), stop=(kvc == KVC - 1))

                    def norm(hi):
                        po = pos[hi]
                        if stage_po:
                            # one cheap copy frees the po PSUM slot; the
                            # rest of the chain runs out of SBUF off the
                            # attn@V critical path
                            pb = pDen.tile([VW, nqb], hdt, tag="pb", bufs=3,
                                           name="pb")
                            nc.vector.tensor_copy(pb[:], po[0:VW, :])
                            src, den_src = pb, pb[64:65, :]
                        else:
                            src, den_src = po, po[64:65, :]
                        d = pDen.tile(
                            [1, nqb], hdt if (norm_pe and not stage_po)
                            else f32, tag="d")
                        if norm_pe and not stage_po:
                            with nc.allow_low_precision(
                                    reason="1/den broadcast operand; bf16 "
                                           "rounding is within the rel-err "
                                           "budget"):
                                nc.vector.reciprocal(d[:], den_src)
                        else:
                            nc.vector.reciprocal(d[:], den_src)
                        R = pDen.tile([64, nqb], f32, tag="r")
                        if norm_pe and not stage_po:
                            # broadcast 1/den across partitions via a PE
                            # outer product into the unused half of po's
                            # bank, then stage to SBUF (DVE reads at most
                            # one PSUM operand)
                            nc.tensor.matmul(po[64:128, :], ones_sb[:], d[:],
                                             start=True, stop=True)
                            nc.vector.tensor_copy(R[:], po[64:128, :])
                        else:
                            nc.gpsimd.partition_broadcast(R[:], d[:])
                        nc.vector.tensor_mul(
                            OT[hi * 64:(hi + 1) * 64,
                               m * NQ + qo:m * NQ + qo + nqb],
                            src[0:64, :], R[:])

                    for kvg in range(NG):
                        s1 = psS.tile([128, cpt * nqb], f32, tag="s")
                        s2 = psS.tile([128, cpt * nqb], f32, tag="s")
                        for j in range(cpt):
                            kvc = kvg * cpt + j
                            ko = m * NKV + kvc * 128
                            kw = dict(start=True, stop=True)
                            if pack_s:
                                kw1 = dict(tile_position=(0, 0), **kw)
                                kw2 = dict(tile_position=(64, 0), **kw)
                            else:
                                kw1 = kw2 = kw
                            nc.tensor.matmul(
                                s1[:, j * nqb:(j + 1) * nqb],
                                KT[0:64, ko:ko + 128],
                                QT[0:64, m * NQ + qo:m * NQ + qo + nqb], **kw1)
                            nc.tensor.matmul(
                                s2[:, j * nqb:(j + 1) * nqb],
                                KT[64:128, ko:ko + 128],
                                QT[64:128, m * NQ + qo:m * NQ + qo + nqb], **kw2)
                        for hi, s in ((0, s1), (1, s2)):
                            pquad = pP.tile([128, cpt * nqb], hdt, tag="pq")
                            nc.scalar.activation(pquad[:], s[:], EXP)
                            quads[hi].append(pquad)
                        if bv and q == 0 and kvg == 0:
                            emit_BV()
                        if kvg > 0:
                            av_chunks(kvg - 1)
                    av_chunks(NG - 1)
                    norm(0)
                    norm(1)

            def emit_C(m, parts="full"):
                do_exp = parts in ("se", "sea", "full")
                do_av = parts in ("sea", "full", "noexp")
                do_norm = parts in ("full", "noexp")
                for q in range(nqblks):
                    qo = q * nqb
                    quads = ([], [])
                    for kvg in range(KVC // cpt):
                        s1 = psS.tile([128, cpt * nqb], f32, tag="s")
                        s2 = psS.tile([128, cpt * nqb], f32, tag="s")
                        for j in range(cpt):
                            kvc = kvg * cpt + j
                            ko = m * NKV + kvc * 128
                            kw = dict(start=True, stop=True)
                            if pack_s:
                                kw1 = dict(tile_position=(0, 0), **kw)
                                kw2 = dict(tile_position=(64, 0), **kw)
                            else:
                                kw1 = kw2 = kw
                            nc.tensor.matmul(
                                s1[:, j * nqb:(j + 1) * nqb],
                                KT[0:64, ko:ko + 128],
                                QT[0:64, m * NQ + qo:m * NQ + qo + nqb], **kw1)
                            nc.tensor.matmul(
                                s2[:, j * nqb:(j + 1) * nqb],
                                KT[64:128, ko:ko + 128],
                                QT[64:128, m * NQ + qo:m * NQ + qo + nqb], **kw2)
                        if not (do_exp or do_av):
                            continue
                        for hi, s in ((0, s1), (1, s2)):
                            pquad = pP.tile([128, cpt * nqb], hdt, tag="pq")
                            if do_exp:
                                nc.scalar.activation(pquad[:], s[:], EXP)
                            quads[hi].append(pquad)
                    if not do_av:
                        continue
                    for hi in range(2):
                        h = 2 * m + hi
                        po = psO.tile([VW, nqb], f32)
                        for kvc in range(KVC):
                            nc.tensor.matmul(
                                po[:],
                                V[:, kvc * HPC * VW + h * VW:
                                   kvc * HPC * VW + (h + 1) * VW],
                                quads[hi][kvc // cpt][:, (kvc % cpt) * nqb:
                                                      (kvc % cpt + 1) * nqb],
                                start=(kvc == 0), stop=(kvc == KVC - 1))
                        if not do_norm:
                            continue
                        d = pDen.tile([1, nqb], f32, tag="d")
                        nc.vector.reciprocal(d[:], po[64:65, :])
                        R = pDen.tile([64, nqb], f32, tag="r")
                        nc.gpsimd.partition_broadcast(R[:], d[:])
                        nc.vector.tensor_mul(
                            OT[hi * 64:(hi + 1) * 64, m * NQ + qo:m * NQ + qo + nqb],
                            po[0:64, :], R[:])

            def emit_D_range(pOutp, mqs):
                for mq in mqs:
                    for n2 in range(OUT_DIM // 512):
                        ps = psAB.tile([128, 512], f32, tag="ab")
                        for c in range(MC):
                            nc.tensor.matmul(
                                ps[:],
                                OT[:, c * NQ + mq * 128:c * NQ + (mq + 1) * 128],
                                wo[:, c * OUT_DIM + n2 * 512:c * OUT_DIM + (n2 + 1) * 512],
                                start=(c == 0), stop=(c == MC - 1))
                        ob = pOutp.tile([128, 512], f32, tag="ob", name="ob")
                        nc.vector.tensor_copy(ob[:], ps[:])
                        nc.sync.dma_start(
                            out_d[mq * 128:(mq + 1) * 128,
                                  n2 * 512:(n2 + 1) * 512],
                            ob[:])

            def emit_D():
                pOut = tc.tile_pool(name="outst", bufs=2)
                with pOut as pOutp:
                    emit_D_range(pOutp, range(NQ // 128))

            if qouter:
                # q-outer: all head-pairs per query block, so phase D for
                # block q-1 is ready while block q computes and overlaps C.
                pOutQ = ctx.enter_context(tc.tile_pool(name="outst", bufs=2))
                mq_per_q = nqb // 128
                for m in range(MC):
                    emit_A(m)
                    emit_BK(m)
                    if m == 0 and not bv_mid:
                        emit_BV()
                    (emit_C_mix if mixexp else emit_C_pipe)(
                        m, bv=(bv_mid and m == 0), qlist=[0])
                for q in range(1, nqblks):
                    for m in range(MC):
                        (emit_C_mix if mixexp else emit_C_pipe)(m, qlist=[q])
                    emit_D_range(pOutQ,
                                 range((q - 1) * mq_per_q, q * mq_per_q))
                emit_D_range(pOutQ,
                             range((nqblks - 1) * mq_per_q, nqblks * mq_per_q))
                return
            for r in range(rep_ab):
                if interleave:
                    for m in range(MC):
                        emit_A(m)
                        emit_BK(m)
                        if m == 0 and not bv_mid:
                            emit_BV()
                        if r == rep_ab - 1:
                            for rc in range(rep_c):
                                if pipec and rc == rep_c - 1:
                                    (emit_C_mix if mixexp else emit_C_pipe)(
                                        m, bv=(bv_mid and m == 0
                                               and rc == rep_c - 1))
                                else:
                                    emit_C(m, parts=(c_parts_all or
                                                     (c_parts if rc < rep_c - 1
                                                      else "full")))
                else:
                    for m in range(MC):
                        emit_A(m)
                    for m in range(MC):
                        emit_BK(m)
                    emit_BV()
                    if r == rep_ab - 1:
                        for rc in range(rep_c):
                            for m in range(MC):
                                if pipec and rc == rep_c - 1:
                                    (emit_C_mix if mixexp else emit_C_pipe)(m)
                                else:
                                    emit_C(m, parts=(c_parts_all or
                                                     (c_parts if rc < rep_c - 1
                                                      else "full")))
            for rd in range(rep_d):
                emit_D()

    with tile.TileContext(nc, trace_sim=False) as tc:
        if reps == 1:
            _emit(tc)
        else:
            with tc.For_i(0, reps, 1):
                _emit(tc)

    nc.compile()
    return nc


def get_program(reps=1, mmdt="bfloat16", **kw):
    key = ("prog", reps, mmdt, tuple(sorted(kw.items())))
    if key not in _CACHE:
        _CACHE[key] = _build_program(reps=reps, mmdt=mmdt, **kw)
    return _CACHE[key]


def make_in_maps(x, context, Wq, Wk, Wv, Wo, mmdt="bfloat16"):
    import ml_dtypes
    hdt = np.float32 if mmdt == "float32r" else np.dtype(ml_dtypes.bfloat16)
    x = np.asarray(x, dtype=np.float32)
    context = np.asarray(context, dtype=np.float32)
    Wq = np.asarray(Wq, dtype=np.float32)
    Wk = np.asarray(Wk, dtype=np.float32)
    Wv = np.asarray(Wv, dtype=np.float32)
    Wo = np.asarray(Wo, dtype=np.float32)
    xT = [np.ascontiguousarray(x[b].T).astype(hdt) for b in range(B)]
    ctxT = [np.ascontiguousarray(context[b].T).astype(hdt) for b in range(B)]
    wq = [(np.ascontiguousarray(Wq[:, g * G:(g + 1) * G]) * np.float32(SCALE))
          .astype(hdt) for g in range(2)]
    wk = [np.ascontiguousarray(Wk[:, g * G:(g + 1) * G]).astype(hdt) for g in range(2)]
    wv = [np.ascontiguousarray(Wv[:, g * G:(g + 1) * G]).astype(hdt) for g in range(2)]
    wo = [np.ascontiguousarray(Wo[g * G:(g + 1) * G, :]).astype(hdt) for g in range(2)]
    in_maps = []
    for c in range(8):
        b, g = c // 2, c % 2
        in_maps.append({"xT": xT[b], "ctxT": ctxT[b], "wq": wq[g],
                        "wk": wk[g], "wv": wv[g], "wo": wo[g]})
    return in_maps


def kernel(x, context, Wq, Wk, Wv, Wo, bo, mmdt="bfloat16", **kw):
    nc = get_program(mmdt=mmdt, **kw)
    in_maps = make_in_maps(x, context, Wq, Wk, Wv, Wo, mmdt=mmdt)
    res = bass_utils.run_bass_kernel_spmd(nc, in_maps, core_ids=list(range(8)))
    bo = np.asarray(bo, dtype=np.float32)
    out = np.empty((B, NQ, OUT_DIM), dtype=np.float32)
    for b in range(B):
        out[b] = res.results[2 * b]["out"] + res.results[2 * b + 1]["out"] + bo
    return out
